# revision 6
# baseline (speedup 1.0000x reference)
"""FISTA sparse-coding (CRsAE dense) Trainium2 Bass kernel.

Problem: T steps of
    r     = x - H @ yk                      # [B, D_in]   residual
    x_new = soft(yk + H^T r / L, lam/L)     # [B, D_enc]  gradient + shrink
    yk    = x_new + c_t (x_new - x_old)     # Nesterov momentum
then z = H @ x_new.

Sharding: data-parallel over batch (2048 -> 8 cores x 256). Each core is
fully independent (no collectives).

Per-core layout (everything transposed: features on partitions, batch on
the free dim, so both GEMMs contract on the partition axis):
    xt  [D_in, BL]   fp32   resident SBUF
    yk  [D_enc, BL]  fp32   resident SBUF (state)
    xo  [D_enc, BL]  fp32   resident SBUF (state, x_old; holds x_new at end)
    Hs = (H/L)       bf16   resident SBUF [D_in, D_enc]  (gradient matmul lhsT)
    Ht = H^T         fp32   streamed from HBM each step  (residual matmul
                            lhsT, used as float32r: full-rate on the PE)

The residual is computed in (near-fp32) float32r, then rounded to bf16 for
the gradient matmul; since r is computed exactly first, its *relative*
accuracy is kept even when FISTA converges and r shrinks.
"""

import numpy as np
import ml_dtypes

import concourse.bacc as bacc
import concourse.mybir as mybir
import concourse.tile as tile
from concourse.bass_utils import run_bass_kernel_spmd

F32 = mybir.dt.float32
F32R = mybir.dt.float32r
BF16 = mybir.dt.bfloat16

L_CONST = 10.0
LAM = 0.1
THETA = LAM / L_CONST  # 0.01

B, DIN, DENC = 2048, 1024, 4096
N_CORES = 8
BL = B // N_CORES  # 256 batch rows per core
P = 128
KM = DIN // P   # 8  D_in partition tiles
KD = DENC // P  # 32 D_enc partition tiles
CH = 2          # Ht k-tiles per streamed chunk (1 MiB per DMA)

_nc_cache = {}


def _momentum_coefs(T):
    """(t_old - 1) / t_new per step, in float32 like the reference."""
    one = np.float32(1.0)
    t_old = np.float32(1.0)
    cs = []
    for _ in range(T):
        t_new = (one + np.sqrt(one + np.float32(4.0) * t_old * t_old)) / np.float32(2.0)
        cs.append(float((t_old - one) / t_new))
        t_old = t_new
    return cs


def _build(T):
    nc = bacc.Bacc(None, target_bir_lowering=False)
    xt = nc.dram_tensor("xt", [DIN, BL], F32, kind="ExternalInput")
    ht = nc.dram_tensor("ht", [DENC, DIN], F32R, kind="ExternalInput")
    hs = nc.dram_tensor("hs", [DIN, DENC], BF16, kind="ExternalInput")
    zt = nc.dram_tensor("zt", [DIN, BL], F32, kind="ExternalOutput")
    xnt = nc.dram_tensor("xnt", [DENC, BL], F32R, kind="ExternalOutput")

    coefs = _momentum_coefs(T)
    Alu = mybir.AluOpType
    RELU = mybir.ActivationFunctionType.Relu

    with tile.TileContext(nc) as tc:
        with (
            tc.tile_pool(name="persist", bufs=1) as pp,
            tc.tile_pool(name="stream", bufs=3) as sp,
            tc.tile_pool(name="tmp", bufs=3) as tp,
            tc.tile_pool(name="rpsum", bufs=4, space="PSUM") as rp,
            tc.tile_pool(name="gpsum", bufs=3, space="PSUM") as gp,
        ):
            negth = pp.tile([P, 1], F32, tag="negth")
            nc.gpsimd.memset(negth[:], -THETA)
            xt_sb = pp.tile([P, KM, BL], F32, tag="xt_sb")
            hs_sb = pp.tile([P, KM, DENC], BF16, tag="hs_sb")
            yk = pp.tile([P, KD, BL], F32R, tag="yk")
            xo = pp.tile([P, KD, BL], F32R, tag="xo")
            rt = pp.tile([P, KM, BL], BF16, tag="rt")
            zsb = pp.tile([P, KM, BL], F32, tag="zsb")

            nc.sync.dma_start(out=xt_sb[:], in_=xt.rearrange("(o p) f -> p o f", p=P))
            nc.sync.dma_start(out=hs_sb[:], in_=hs.rearrange("(o p) f -> p o f", p=P))

            ht_r = ht.rearrange("(o p) f -> p o f", p=P)  # [128, KD, DIN]

            def residual_matmuls(rhs_state):
                """psum[m] = (H @ state)^T tiles, contracting D_enc.

                Streams Ht from HBM in CH-k-tile chunks; returns KM//2 psum
                tiles [P, 2*BL] (two D_in tiles packed per PSUM bank).
                """
                psums = [
                    rp.tile([P, 2 * BL], F32, tag="rps", name=f"rps{j}")
                    for j in range(KM // 2)
                ]
                for c in range(KD // CH):
                    htt = sp.tile([P, CH, DIN], F32R, tag="htt")
                    nc.sync.dma_start(out=htt[:], in_=ht_r[:, c * CH:(c + 1) * CH, :])
                    for j in range(CH):
                        k = c * CH + j
                        lt = htt[:, j, :]
                        mv = rhs_state[:, k, :]
                        for m in range(KM):
                            # Two regions share each PSUM bank. start=True
                            # clears has_written for the WHOLE bank, so only
                            # the first matmul touching the bank may set it;
                            # the second region's first write overwrites via
                            # the cleared has_written bit instead.
                            nc.tensor.matmul(
                                psums[m // 2][:, (m % 2) * BL:(m % 2 + 1) * BL],
                                lt[:, m * P:(m + 1) * P],
                                mv,
                                start=(k == 0 and m % 2 == 0),
                                stop=(k == KD - 1),
                                skip_group_check=True,
                            )
                return psums

            def phase2(t):
                """x_new = soft(yk + psum), momentum update, in-place state."""
                first = t == 0
                cm = coefs[t]
                for mp in range(KD // 2):
                    ps = gp.tile([P, 2 * BL], F32, tag="gps")
                    for half in range(2):
                        m = 2 * mp + half
                        out_ap = ps[:, half * BL:(half + 1) * BL]
                        for k in range(KM):
                            nc.tensor.matmul(
                                out_ap,
                                hs_sb[:, k, m * P:(m + 1) * P],
                                rt[:, k, :],
                                start=(k == 0),
                                stop=(k == KM - 1),
                            )
                    for half in range(2):
                        m = 2 * mp + half
                        pg = ps[:, half * BL:(half + 1) * BL]
                        p_t = tp.tile([P, BL], F32, tag="p_t")
                        n_t = tp.tile([P, BL], F32, tag="n_t")
                        if first:
                            src = pg
                        else:
                            tmp = tp.tile([P, BL], F32, tag="tmp")
                            nc.vector.tensor_tensor(tmp[:], yk[:, m, :], pg, Alu.add)
                            src = tmp[:]
                        nc.scalar.activation(p_t[:], src, RELU, bias=negth[:], scale=1.0)
                        nc.scalar.activation(n_t[:], src, RELU, bias=negth[:], scale=-1.0)
                        if first:
                            # c_0 == 0: yk = x_new = x_old
                            nc.vector.tensor_tensor(xo[:, m, :], p_t[:], n_t[:], Alu.subtract)
                            nc.scalar.copy(yk[:, m, :], xo[:, m, :])
                        else:
                            xs = tp.tile([P, BL], F32, tag="xs")
                            nc.scalar.mul(xs[:], xo[:, m, :], cm)
                            nc.vector.tensor_tensor(xo[:, m, :], p_t[:], n_t[:], Alu.subtract)
                            nc.vector.scalar_tensor_tensor(
                                yk[:, m, :], xo[:, m, :], 1.0 + cm, xs[:],
                                Alu.mult, Alu.subtract,
                            )

            # t = 0: yk == 0 so r = x directly.
            nc.vector.tensor_copy(out=rt[:], in_=xt_sb[:])
            phase2(0)
            for t in range(1, T):
                psums = residual_matmuls(yk)
                for j in range(KM // 2):
                    nc.vector.tensor_tensor(
                        rt[:, 2 * j:2 * j + 2, :],
                        xt_sb[:, 2 * j:2 * j + 2, :],
                        psums[j].rearrange("p (a f) -> p a f", f=BL),
                        Alu.subtract,
                    )
                phase2(t)

            # z = H @ x_new  (x_new lives in xo after the last step)
            psums = residual_matmuls(xo)
            for j in range(KM // 2):
                nc.vector.tensor_copy(
                    out=zsb[:, 2 * j:2 * j + 2, :],
                    in_=psums[j].rearrange("p (a f) -> p a f", f=BL),
                )
            nc.sync.dma_start(out=zt.rearrange("(o p) f -> p o f", p=P), in_=zsb[:])
            nc.sync.dma_start(out=xnt.rearrange("(o p) f -> p o f", p=P), in_=xo[:])

    nc.finalize()
    return nc


def _get_nc(T):
    nc = _nc_cache.get(T)
    if nc is None:
        nc = _build(T)
        _nc_cache[T] = nc
    return nc


def kernel(x=None, H=None, T=None, trace=False, **_ignored):
    x = np.asarray(x, dtype=np.float32)
    H = np.asarray(H, dtype=np.float32)
    T = int(np.asarray(T))
    assert x.shape == (B, DIN, 1), x.shape
    assert H.shape == (DIN, DENC), H.shape

    if T <= 0:
        z = np.zeros((B, DIN, 1), np.float32)
        xn = np.zeros((B, DENC, 1), np.float32)
        return z, xn

    nc = _get_nc(T)

    x2 = x[:, :, 0]                                    # [B, D_in]
    ht_host = np.ascontiguousarray(H.T)                # [D_enc, D_in] fp32
    hs_host = np.ascontiguousarray(
        (H * np.float32(1.0 / L_CONST)).astype(ml_dtypes.bfloat16)
    )                                                  # [D_in, D_enc] bf16

    in_maps = []
    for c in range(N_CORES):
        xt_c = np.ascontiguousarray(x2[c * BL:(c + 1) * BL].T)  # [D_in, BL]
        in_maps.append({"xt": xt_c, "ht": ht_host, "hs": hs_host})

    res = run_bass_kernel_spmd(
        nc, in_maps, core_ids=list(range(N_CORES)), trace=trace
    )

    z = np.empty((B, DIN, 1), np.float32)
    xn = np.empty((B, DENC, 1), np.float32)
    for c in range(N_CORES):
        z[c * BL:(c + 1) * BL, :, 0] = res.results[c]["zt"].T
        xn[c * BL:(c + 1) * BL, :, 0] = res.results[c]["xnt"].T

    if trace:
        kernel.last_results = res
    return z, xn


kernel.last_results = None


# revision 10
# speedup vs baseline: 1.0195x; 1.0195x over previous
"""FISTA sparse-coding (CRsAE dense) Trainium2 Bass kernel.

Problem: T steps of
    r     = x - H @ yk                      # [B, D_in]   residual
    x_new = soft(yk + H^T r / L, lam/L)     # [B, D_enc]  gradient + shrink
    yk    = x_new + c_t (x_new - x_old)     # Nesterov momentum
then z = H @ x_new.

Sharding: data-parallel over batch (2048 -> 8 cores x 256). Each core is
fully independent (no collectives).

Per-core layout (everything transposed: features on partitions, batch on
the free dim, so both GEMMs contract on the partition axis):
    xt  [D_in, BL]   fp32   resident SBUF (also reused as the z buffer)
    yk  [D_enc, BL]  fp32   resident SBUF (state)
    ykb [D_enc, BL]  bf16   resident SBUF (bf16 shadow of yk for the PE)
    xo  [D_enc, BL]  f32r   resident SBUF (state x_old; holds x_new at end)
    Hs = (H/L)       bf16   resident SBUF [D_in, D_enc]  (gradient matmul)
    Htb = H^T        bf16   streamed from HBM each step  (residual matmul;
                            8 MB/step keeps DMA and PE balanced)
    Ht  = H^T        f32r   streamed once at the end for an accurate z pass

The residual r is computed with bf16 weights (error is dominated by the
bf16 gradient matmul anyway), then rounded to bf16 for the gradient
matmul. The final reconstruction z = H @ x_new runs in float32r
(TF32-like, near-fp32) for output accuracy.
"""

import os

import numpy as np
import ml_dtypes

import concourse.bacc as bacc
import concourse.mybir as mybir
import concourse.tile as tile
from concourse.bass_utils import run_bass_kernel_spmd

F32 = mybir.dt.float32
F32R = mybir.dt.float32r
BF16 = mybir.dt.bfloat16

L_CONST = 10.0
LAM = 0.1
THETA = LAM / L_CONST  # 0.01

B, DIN, DENC = 2048, 1024, 4096
N_CORES = 8
BL = B // N_CORES  # 256 batch rows per core
P = 128
KM = DIN // P   # 8  D_in partition tiles
KD = DENC // P  # 32 D_enc partition tiles
CHB = 4         # bf16 Ht k-tiles per streamed chunk (1 MiB per DMA)
CHF = 2         # f32 Ht k-tiles per streamed chunk (1 MiB per DMA)

_nc_cache = {}


def _momentum_coefs(T):
    """(t_old - 1) / t_new per step, in float32 like the reference."""
    one = np.float32(1.0)
    t_old = np.float32(1.0)
    cs = []
    for _ in range(T):
        t_new = (one + np.sqrt(one + np.float32(4.0) * t_old * t_old)) / np.float32(2.0)
        cs.append(float((t_old - one) / t_new))
        t_old = t_new
    return cs


def _build(T):
    nc = bacc.Bacc(None, target_bir_lowering=False)
    xt = nc.dram_tensor("xt", [DIN, BL], F32, kind="ExternalInput")
    ht = nc.dram_tensor("ht", [DENC, DIN], F32R, kind="ExternalInput")
    htb = nc.dram_tensor("htb", [DENC, DIN], BF16, kind="ExternalInput")
    hs = nc.dram_tensor("hs", [DIN, DENC], BF16, kind="ExternalInput")
    zt = nc.dram_tensor("zt", [DIN, BL], F32, kind="ExternalOutput")
    xnt = nc.dram_tensor("xnt", [DENC, BL], F32R, kind="ExternalOutput")

    coefs = _momentum_coefs(T)
    Alu = mybir.AluOpType
    RELU = mybir.ActivationFunctionType.Relu

    with tile.TileContext(nc) as tc:
        with (
            tc.tile_pool(name="persist", bufs=1) as pp,
            tc.tile_pool(name="stream", bufs=2) as sp,
            tc.tile_pool(name="tmp", bufs=2) as tp,
            tc.tile_pool(name="rpsum", bufs=4, space="PSUM") as rp,
            tc.tile_pool(name="gpsum", bufs=4, space="PSUM") as gp,
        ):
            negth = pp.tile([P, 1], F32, tag="negth")
            nc.gpsimd.memset(negth[:], -THETA)
            xt_sb = pp.tile([P, KM, BL], F32, tag="xt_sb")
            hs_sb = pp.tile([P, KM, DENC], BF16, tag="hs_sb")
            yk = pp.tile([P, KD, BL], F32, tag="yk")
            ykb = pp.tile([P, KD, BL], BF16, tag="ykb")
            xo = pp.tile([P, KD, BL], F32R, tag="xo")
            rt = pp.tile([P, KM, BL], BF16, tag="rt")

            nc.sync.dma_start(out=xt_sb[:], in_=xt.rearrange("(o p) f -> p o f", p=P))
            nc.sync.dma_start(out=hs_sb[:], in_=hs.rearrange("(o p) f -> p o f", p=P))

            ht_r = ht.rearrange("(o p) f -> p o f", p=P)    # [128, KD, DIN] f32r
            htb_r = htb.rearrange("(o p) f -> p o f", p=P)  # [128, KD, DIN] bf16

            def residual_matmuls(rhs_state, bf):
                """psum[m] = (H @ state)^T tiles, contracting D_enc.

                Streams Ht from HBM; returns KM//2 psum tiles [P, 2*BL]
                (two D_in tiles packed per PSUM bank). bf=True uses the
                bf16 weights (8 MB/step), else float32r (16 MB, z pass).
                """
                psums = [
                    rp.tile([P, 2 * BL], F32, tag="rps", name=f"rps{j}")
                    for j in range(KM // 2)
                ]
                ch = CHB if bf else CHF
                src = htb_r if bf else ht_r
                dt_ = BF16 if bf else F32R
                for c in range(KD // ch):
                    # bf16 [P, 4, DIN] and f32 [P, 2, DIN] chunks are both
                    # 8 KiB/partition; one tag so they share pool slots.
                    htt = sp.tile([P, ch, DIN], dt_, tag="htt")
                    nc.sync.dma_start(out=htt[:], in_=src[:, c * ch:(c + 1) * ch, :])
                    for j in range(ch):
                        k = c * ch + j
                        lt = htt[:, j, :]
                        mv = rhs_state[:, k, :]
                        for m in range(KM):
                            # Two regions share each PSUM bank. start=True
                            # clears has_written for the WHOLE bank, so only
                            # the first matmul touching the bank may set it;
                            # the second region's first write overwrites via
                            # the cleared has_written bit instead.
                            nc.tensor.matmul(
                                psums[m // 2][:, (m % 2) * BL:(m % 2 + 1) * BL],
                                lt[:, m * P:(m + 1) * P],
                                mv,
                                start=(k == 0 and m % 2 == 0),
                                stop=(k == KD - 1),
                                skip_group_check=True,
                            )
                return psums

            def phase2(t):
                """x_new = soft(yk + psum), momentum update, in-place state.

                Elementwise ops run on [P, 2, BL] pairs (one PSUM bank's
                worth) to halve instruction count.
                """
                first = t == 0
                cm = coefs[t]
                for mp in range(KD // 2):
                    ps = gp.tile([P, 2 * BL], F32, tag="gps")
                    for half in range(2):
                        m = 2 * mp + half
                        out_ap = ps[:, half * BL:(half + 1) * BL]
                        for k in range(KM):
                            nc.tensor.matmul(
                                out_ap,
                                hs_sb[:, k, m * P:(m + 1) * P],
                                rt[:, k, :],
                                start=(k == 0 and half == 0),
                                stop=(k == KM - 1),
                                skip_group_check=True,
                            )
                    sl = slice(2 * mp, 2 * mp + 2)
                    psp = ps.rearrange("p (a f) -> p a f", f=BL)
                    p_t = tp.tile([P, 2, BL], F32, tag="p_t")
                    n_t = tp.tile([P, 2, BL], F32, tag="n_t")
                    if first:
                        src = psp
                    else:
                        tmp = tp.tile([P, 2, BL], F32, tag="tmp")
                        nc.vector.tensor_tensor(tmp[:], yk[:, sl, :], psp, Alu.add)
                        src = tmp[:]
                    nc.scalar.activation(p_t[:], src, RELU, bias=negth[:], scale=1.0)
                    nc.scalar.activation(n_t[:], src, RELU, bias=negth[:], scale=-1.0)
                    if first:
                        # c_0 == 0: yk = x_new = x_old
                        nc.vector.tensor_tensor(xo[:, sl, :], p_t[:], n_t[:], Alu.subtract)
                        nc.scalar.copy(yk[:, sl, :], xo[:, sl, :])
                    else:
                        xs = tp.tile([P, 2, BL], F32, tag="xs")
                        nc.scalar.mul(xs[:], xo[:, sl, :], cm)
                        nc.vector.tensor_tensor(xo[:, sl, :], p_t[:], n_t[:], Alu.subtract)
                        nc.vector.scalar_tensor_tensor(
                            yk[:, sl, :], xo[:, sl, :], 1.0 + cm, xs[:],
                            Alu.mult, Alu.subtract,
                        )
                    if t < T - 1:
                        # bf16 shadow of yk for the next residual matmul
                        nc.any.tensor_copy(out=ykb[:, sl, :], in_=yk[:, sl, :])

            # Diagnostic knobs (timing experiments only; default off).
            skip_p1 = os.environ.get("KBENCH_SKIP_P1", "0") == "1"
            skip_p2 = os.environ.get("KBENCH_SKIP_P2", "0") == "1"
            reps = int(os.environ.get("KBENCH_REPS", "0"))

            def body():
                # t = 0: yk == 0 so r = x directly.
                nc.vector.tensor_copy(out=rt[:], in_=xt_sb[:])
                phase2(0)
                for t in range(1, T):
                    if not skip_p1:
                        psums = residual_matmuls(ykb, bf=True)
                        for j in range(KM // 2):
                            nc.vector.tensor_tensor(
                                rt[:, 2 * j:2 * j + 2, :],
                                xt_sb[:, 2 * j:2 * j + 2, :],
                                psums[j].rearrange("p (a f) -> p a f", f=BL),
                                Alu.subtract,
                            )
                    if not skip_p2:
                        phase2(t)

                # z = H @ x_new in float32r (x_new lives in xo); xt_sb is
                # dead by now and reused as the z staging buffer.
                psums = residual_matmuls(xo, bf=False)
                for j in range(KM // 2):
                    nc.vector.tensor_copy(
                        out=xt_sb[:, 2 * j:2 * j + 2, :],
                        in_=psums[j].rearrange("p (a f) -> p a f", f=BL),
                    )
                nc.sync.dma_start(out=zt.rearrange("(o p) f -> p o f", p=P), in_=xt_sb[:])
                nc.sync.dma_start(out=xnt.rearrange("(o p) f -> p o f", p=P), in_=xo[:])

            if reps > 0:
                with tc.For_i(0, reps):
                    body()
            else:
                body()

    nc.finalize()
    return nc


def _get_nc(T):
    nc = _nc_cache.get(T)
    if nc is None:
        nc = _build(T)
        _nc_cache[T] = nc
    return nc


def kernel(x=None, H=None, T=None, trace=False, **_ignored):
    x = np.asarray(x, dtype=np.float32)
    H = np.asarray(H, dtype=np.float32)
    T = int(np.asarray(T))
    assert x.shape == (B, DIN, 1), x.shape
    assert H.shape == (DIN, DENC), H.shape

    if T <= 0:
        z = np.zeros((B, DIN, 1), np.float32)
        xn = np.zeros((B, DENC, 1), np.float32)
        return z, xn

    nc = _get_nc(T)

    x2 = x[:, :, 0]                                    # [B, D_in]
    ht_host = np.ascontiguousarray(H.T)                # [D_enc, D_in] fp32
    htb_host = np.ascontiguousarray(H.T.astype(ml_dtypes.bfloat16))
    hs_host = np.ascontiguousarray(
        (H * np.float32(1.0 / L_CONST)).astype(ml_dtypes.bfloat16)
    )                                                  # [D_in, D_enc] bf16

    in_maps = []
    for c in range(N_CORES):
        xt_c = np.ascontiguousarray(x2[c * BL:(c + 1) * BL].T)  # [D_in, BL]
        in_maps.append({"xt": xt_c, "ht": ht_host, "htb": htb_host, "hs": hs_host})

    res = run_bass_kernel_spmd(
        nc, in_maps, core_ids=list(range(N_CORES)), trace=trace
    )

    z = np.empty((B, DIN, 1), np.float32)
    xn = np.empty((B, DENC, 1), np.float32)
    for c in range(N_CORES):
        z[c * BL:(c + 1) * BL, :, 0] = res.results[c]["zt"].T
        xn[c * BL:(c + 1) * BL, :, 0] = res.results[c]["xnt"].T

    if trace:
        kernel.last_results = res
    return z, xn


kernel.last_results = None


# revision 11
# speedup vs baseline: 1.1786x; 1.1560x over previous
"""FISTA sparse-coding (CRsAE dense) Trainium2 Bass kernel.

Problem: T steps of
    r     = x - H @ yk                      # [B, D_in]   residual
    x_new = soft(yk + H^T r / L, lam/L)     # [B, D_enc]  gradient + shrink
    yk    = x_new + c_t (x_new - x_old)     # Nesterov momentum
then z = H @ x_new.

Sharding: data-parallel over batch (2048 -> 8 cores x 256). Each core is
fully independent (no collectives).

Per-core layout (everything transposed: features on partitions, batch on
the free dim, so both GEMMs contract on the partition axis):
    xt  [D_in, BL]   fp32  resident SBUF (also reused as the z buffer)
    yk  [D_enc, BL]  fp32  resident SBUF (state)
    ykb [D_enc, BL]  bf16  resident SBUF (bf16 shadow of yk for the PE;
                           holds bf16(x_new) for the final z pass)
    xo  [D_enc, BL]  fp32  resident SBUF (state x_old; holds x_new at end)
    Hs = (H/L)       bf16  resident SBUF [D_in, D_enc]  (gradient matmul)
    Htb = H^T        bf16  streamed from HBM each step  (residual matmul;
                           8 MB/step keeps DMA and PE balanced)

All matmuls are bf16 x bf16 -> fp32 PSUM. The state and all elementwise
arithmetic stay fp32; the residual r is computed from bf16-rounded
operands and rounded to bf16 afterwards, which keeps its relative
accuracy. Elementwise work runs on [128, 2, 256] pairs (one PSUM bank)
to halve instruction count.
"""

import os

import numpy as np
import ml_dtypes

import concourse.bacc as bacc
import concourse.mybir as mybir
import concourse.tile as tile
from concourse.bass_utils import run_bass_kernel_spmd

F32 = mybir.dt.float32
BF16 = mybir.dt.bfloat16

L_CONST = 10.0
LAM = 0.1
THETA = LAM / L_CONST  # 0.01

B, DIN, DENC = 2048, 1024, 4096
N_CORES = 8
BL = B // N_CORES  # 256 batch rows per core
P = 128
KM = DIN // P   # 8  D_in partition tiles
KD = DENC // P  # 32 D_enc partition tiles
CH = 2          # Htb k-tiles per streamed chunk (1 MiB per DMA)

_nc_cache = {}


def _momentum_coefs(T):
    """(t_old - 1) / t_new per step, in float32 like the reference."""
    one = np.float32(1.0)
    t_old = np.float32(1.0)
    cs = []
    for _ in range(T):
        t_new = (one + np.sqrt(one + np.float32(4.0) * t_old * t_old)) / np.float32(2.0)
        cs.append(float((t_old - one) / t_new))
        t_old = t_new
    return cs


def _build(T):
    nc = bacc.Bacc(None, target_bir_lowering=False)
    xt = nc.dram_tensor("xt", [DIN, BL], F32, kind="ExternalInput")
    htb = nc.dram_tensor("htb", [DENC, DIN], BF16, kind="ExternalInput")
    hs = nc.dram_tensor("hs", [DIN, DENC], BF16, kind="ExternalInput")
    zt = nc.dram_tensor("zt", [DIN, BL], F32, kind="ExternalOutput")
    xnt = nc.dram_tensor("xnt", [DENC, BL], F32, kind="ExternalOutput")

    coefs = _momentum_coefs(T)
    Alu = mybir.AluOpType
    RELU = mybir.ActivationFunctionType.Relu

    with tile.TileContext(nc) as tc:
        with (
            tc.tile_pool(name="persist", bufs=1) as pp,
            tc.tile_pool(name="stream", bufs=6) as sp,
            tc.tile_pool(name="tmp", bufs=2) as tp,
            tc.tile_pool(name="xsp", bufs=1) as xp,
            tc.tile_pool(name="rpsum", bufs=4, space="PSUM") as rp,
            tc.tile_pool(name="gpsum", bufs=3, space="PSUM") as gp,
        ):
            negth = pp.tile([P, 1], F32, tag="negth")
            nc.gpsimd.memset(negth[:], -THETA)
            xt_sb = pp.tile([P, KM, BL], F32, tag="xt_sb")
            hs_sb = pp.tile([P, KM, DENC], BF16, tag="hs_sb")
            yk = pp.tile([P, KD, BL], F32, tag="yk")
            ykb = pp.tile([P, KD, BL], BF16, tag="ykb")
            xo = pp.tile([P, KD, BL], F32, tag="xo")
            rt = pp.tile([P, KM, BL], BF16, tag="rt")

            nc.sync.dma_start(out=xt_sb[:], in_=xt.rearrange("(o p) f -> p o f", p=P))
            nc.sync.dma_start(out=hs_sb[:], in_=hs.rearrange("(o p) f -> p o f", p=P))

            htb_r = htb.rearrange("(o p) f -> p o f", p=P)  # [128, KD, DIN] bf16

            def residual_matmuls():
                """psum[m] = (H @ ykb)^T tiles, contracting D_enc.

                Streams Htb (bf16) from HBM in 1 MiB chunks; returns KM//2
                psum tiles [P, 2*BL] (two D_in tiles per PSUM bank).
                """
                psums = [
                    rp.tile([P, 2 * BL], F32, tag="rps", name=f"rps{j}")
                    for j in range(KM // 2)
                ]
                for c in range(KD // CH):
                    htt = sp.tile([P, CH, DIN], BF16, tag="htt")
                    nc.sync.dma_start(out=htt[:], in_=htb_r[:, c * CH:(c + 1) * CH, :])
                    for j in range(CH):
                        k = c * CH + j
                        lt = htt[:, j, :]
                        mv = ykb[:, k, :]
                        for m in range(KM):
                            # Two regions share each PSUM bank. start=True
                            # clears has_written for the WHOLE bank, so only
                            # the first matmul touching the bank may set it;
                            # the other region's first write overwrites via
                            # the cleared has_written bit instead.
                            nc.tensor.matmul(
                                psums[m // 2][:, (m % 2) * BL:(m % 2 + 1) * BL],
                                lt[:, m * P:(m + 1) * P],
                                mv,
                                start=(k == 0 and m % 2 == 0),
                                stop=(k == KD - 1),
                                skip_group_check=True,
                            )
                return psums

            def phase2(t):
                """x_new = soft(yk + psum), momentum update, in-place state."""
                first = t == 0
                last = t == T - 1
                cm = coefs[t]
                for mp in range(KD // 2):
                    ps = gp.tile([P, 2 * BL], F32, tag="gps")
                    for half in range(2):
                        m = 2 * mp + half
                        out_ap = ps[:, half * BL:(half + 1) * BL]
                        for k in range(KM):
                            nc.tensor.matmul(
                                out_ap,
                                hs_sb[:, k, m * P:(m + 1) * P],
                                rt[:, k, :],
                                start=(k == 0 and half == 0),
                                stop=(k == KM - 1),
                                skip_group_check=True,
                            )
                    sl = slice(2 * mp, 2 * mp + 2)
                    psp = ps.rearrange("p (a f) -> p a f", f=BL)
                    if not first:
                        # t = yk + G/L, in place in PSUM
                        nc.vector.tensor_tensor(psp, yk[:, sl, :], psp, Alu.add)
                    p_t = tp.tile([P, 2, BL], F32, tag="p_t")
                    n_t = tp.tile([P, 2, BL], F32, tag="n_t")
                    nc.scalar.activation(p_t[:], psp, RELU, bias=negth[:], scale=1.0)
                    nc.scalar.activation(n_t[:], psp, RELU, bias=negth[:], scale=-1.0)
                    if first or last:
                        # first: c_0 == 0 so yk = x_new; last: yk_T unused.
                        nc.vector.tensor_tensor(xo[:, sl, :], p_t[:], n_t[:], Alu.subtract)
                        if not last:
                            nc.scalar.copy(yk[:, sl, :], xo[:, sl, :])
                            nc.vector.tensor_copy(out=ykb[:, sl, :], in_=xo[:, sl, :])
                        else:
                            # bf16(x_new) for the final z = H @ x_new pass
                            nc.vector.tensor_copy(out=ykb[:, sl, :], in_=xo[:, sl, :])
                    else:
                        xs = xp.tile([P, 2, BL], F32, tag="xs")
                        nc.scalar.mul(xs[:], xo[:, sl, :], cm)
                        nc.vector.tensor_tensor(xo[:, sl, :], p_t[:], n_t[:], Alu.subtract)
                        nc.vector.scalar_tensor_tensor(
                            yk[:, sl, :], xo[:, sl, :], 1.0 + cm, xs[:],
                            Alu.mult, Alu.subtract,
                        )
                        # bf16 shadow of yk for the next residual matmul,
                        # emitted right after so next phase 1 starts early
                        nc.vector.tensor_copy(out=ykb[:, sl, :], in_=yk[:, sl, :])

            # Diagnostic knobs (timing experiments only; default off).
            skip_p1 = os.environ.get("KBENCH_SKIP_P1", "0") == "1"
            skip_p2 = os.environ.get("KBENCH_SKIP_P2", "0") == "1"
            reps = int(os.environ.get("KBENCH_REPS", "0"))

            def body():
                # t = 0: yk == 0 so r = x directly.
                nc.vector.tensor_copy(out=rt[:], in_=xt_sb[:])
                phase2(0)
                for t in range(1, T):
                    if not skip_p1:
                        psums = residual_matmuls()
                        for j in range(KM // 2):
                            nc.vector.tensor_tensor(
                                rt[:, 2 * j:2 * j + 2, :],
                                xt_sb[:, 2 * j:2 * j + 2, :],
                                psums[j].rearrange("p (a f) -> p a f", f=BL),
                                Alu.subtract,
                            )
                    if not skip_p2:
                        phase2(t)

                # z = H @ x_new (ykb holds bf16(x_new) now); xt_sb is dead
                # and reused as the z staging buffer.
                psums = residual_matmuls()
                for j in range(KM // 2):
                    nc.vector.tensor_copy(
                        out=xt_sb[:, 2 * j:2 * j + 2, :],
                        in_=psums[j].rearrange("p (a f) -> p a f", f=BL),
                    )
                nc.sync.dma_start(out=zt.rearrange("(o p) f -> p o f", p=P), in_=xt_sb[:])
                nc.sync.dma_start(out=xnt.rearrange("(o p) f -> p o f", p=P), in_=xo[:])

            if reps > 0:
                with tc.For_i(0, reps):
                    body()
            else:
                body()

    nc.finalize()
    return nc


def _get_nc(T):
    nc = _nc_cache.get(T)
    if nc is None:
        nc = _build(T)
        _nc_cache[T] = nc
    return nc


def kernel(x=None, H=None, T=None, trace=False, **_ignored):
    x = np.asarray(x, dtype=np.float32)
    H = np.asarray(H, dtype=np.float32)
    T = int(np.asarray(T))
    assert x.shape == (B, DIN, 1), x.shape
    assert H.shape == (DIN, DENC), H.shape

    if T <= 0:
        z = np.zeros((B, DIN, 1), np.float32)
        xn = np.zeros((B, DENC, 1), np.float32)
        return z, xn

    nc = _get_nc(T)

    x2 = x[:, :, 0]                                    # [B, D_in]
    htb_host = np.ascontiguousarray(H.T.astype(ml_dtypes.bfloat16))
    hs_host = np.ascontiguousarray(
        (H * np.float32(1.0 / L_CONST)).astype(ml_dtypes.bfloat16)
    )                                                  # [D_in, D_enc] bf16

    in_maps = []
    for c in range(N_CORES):
        xt_c = np.ascontiguousarray(x2[c * BL:(c + 1) * BL].T)  # [D_in, BL]
        in_maps.append({"xt": xt_c, "htb": htb_host, "hs": hs_host})

    res = run_bass_kernel_spmd(
        nc, in_maps, core_ids=list(range(N_CORES)), trace=trace
    )

    z = np.empty((B, DIN, 1), np.float32)
    xn = np.empty((B, DENC, 1), np.float32)
    for c in range(N_CORES):
        z[c * BL:(c + 1) * BL, :, 0] = res.results[c]["zt"].T
        xn[c * BL:(c + 1) * BL, :, 0] = res.results[c]["xnt"].T

    if trace:
        kernel.last_results = res
    return z, xn


kernel.last_results = None


# revision 13
# speedup vs baseline: 5890.9224x; 4998.3854x over previous
"""FISTA sparse-coding (CRsAE dense) Trainium2 Bass kernel.

Problem: T steps of
    r     = x - H @ yk                      # [B, D_in]   residual
    x_new = soft(yk + H^T r / L, lam/L)     # [B, D_enc]  gradient + shrink
    yk    = x_new + c_t (x_new - x_old)     # Nesterov momentum
then z = H @ x_new.

Sharding: data-parallel over batch (2048 -> 8 cores x 256). Each core is
fully independent (no collectives).

Per-core layout (everything transposed: features on partitions, batch on
the free dim, so both GEMMs contract on the partition axis):
    xt  [D_in, BL]   fp32  resident SBUF (also reused as the z buffer)
    yk  [D_enc, BL]  fp32  resident SBUF (state)
    ykb [D_enc, BL]  bf16  resident SBUF (bf16 shadow of yk for the PE;
                           holds bf16(x_new) for the final z pass)
    xo  [D_enc, BL]  fp32  resident SBUF (state x_old; holds x_new at end)
    Hs = (H/L)       bf16  resident SBUF [D_in, D_enc]  (gradient matmul)
    Htb = H^T        bf16  streamed from HBM each step  (residual matmul;
                           8 MB/step keeps DMA and PE balanced)

All matmuls are bf16 x bf16 -> fp32 PSUM. The state and all elementwise
arithmetic stay fp32; the residual r is computed from bf16-rounded
operands and rounded to bf16 afterwards, which keeps its relative
accuracy. Elementwise work runs on [128, 2, 256] pairs (one PSUM bank)
to halve instruction count.
"""

import os

import numpy as np
import ml_dtypes

import concourse.bacc as bacc
import concourse.mybir as mybir
import concourse.tile as tile
from concourse.bass_utils import run_bass_kernel_spmd

F32 = mybir.dt.float32
BF16 = mybir.dt.bfloat16

L_CONST = 10.0
LAM = 0.1
THETA = LAM / L_CONST  # 0.01

B, DIN, DENC = 2048, 1024, 4096
N_CORES = 8
BL = B // N_CORES  # 256 batch rows per core
P = 128
KM = DIN // P   # 8  D_in partition tiles
KD = DENC // P  # 32 D_enc partition tiles
CH = 2          # Htb k-tiles per streamed chunk (1 MiB per DMA)

_nc_cache = {}


def _momentum_coefs(T):
    """(t_old - 1) / t_new per step, in float32 like the reference."""
    one = np.float32(1.0)
    t_old = np.float32(1.0)
    cs = []
    for _ in range(T):
        t_new = (one + np.sqrt(one + np.float32(4.0) * t_old * t_old)) / np.float32(2.0)
        cs.append(float((t_old - one) / t_new))
        t_old = t_new
    return cs


def _build(T):
    nc = bacc.Bacc(None, target_bir_lowering=False)
    xt = nc.dram_tensor("xt", [DIN, BL], F32, kind="ExternalInput")
    htb = nc.dram_tensor("htb", [DENC, DIN], BF16, kind="ExternalInput")
    hs = nc.dram_tensor("hs", [DIN, DENC], BF16, kind="ExternalInput")
    zt = nc.dram_tensor("zt", [DIN, BL], F32, kind="ExternalOutput")
    xnt = nc.dram_tensor("xnt", [DENC, BL], F32, kind="ExternalOutput")

    coefs = _momentum_coefs(T)
    Alu = mybir.AluOpType
    RELU = mybir.ActivationFunctionType.Relu

    with tile.TileContext(nc) as tc:
        with (
            tc.tile_pool(name="persist", bufs=1) as pp,
            tc.tile_pool(name="stream", bufs=6) as sp,
            tc.tile_pool(name="tmp", bufs=2) as tp,
            tc.tile_pool(name="xsp", bufs=1) as xp,
            tc.tile_pool(name="rpsum", bufs=4, space="PSUM") as rp,
            tc.tile_pool(name="gpsum", bufs=3, space="PSUM") as gp,
        ):
            negth = pp.tile([P, 1], F32, tag="negth")
            nc.gpsimd.memset(negth[:], -THETA)
            xt_sb = pp.tile([P, KM, BL], F32, tag="xt_sb")
            hs_sb = pp.tile([P, KM, DENC], BF16, tag="hs_sb")
            yk = pp.tile([P, KD, BL], F32, tag="yk")
            ykb = pp.tile([P, KD, BL], BF16, tag="ykb")
            xo = pp.tile([P, KD, BL], F32, tag="xo")
            rt = pp.tile([P, KM, BL], BF16, tag="rt")

            nc.sync.dma_start(out=xt_sb[:], in_=xt.rearrange("(o p) f -> p o f", p=P))
            nc.sync.dma_start(out=hs_sb[:], in_=hs.rearrange("(o p) f -> p o f", p=P))

            htb_r = htb.rearrange("(o p) f -> p o f", p=P)  # [128, KD, DIN] bf16

            def residual_matmuls():
                """psum[m] = (H @ ykb)^T tiles, contracting D_enc.

                Streams Htb (bf16) from HBM in 1 MiB chunks; returns KM//2
                psum tiles [P, 2*BL] (two D_in tiles per PSUM bank).
                """
                psums = [
                    rp.tile([P, 2 * BL], F32, tag="rps", name=f"rps{j}")
                    for j in range(KM // 2)
                ]
                for c in range(KD // CH):
                    htt = sp.tile([P, CH, DIN], BF16, tag="htt")
                    nc.sync.dma_start(out=htt[:], in_=htb_r[:, c * CH:(c + 1) * CH, :])
                    for j in range(CH):
                        k = c * CH + j
                        lt = htt[:, j, :]
                        mv = ykb[:, k, :]
                        for m in range(KM):
                            # Two regions share each PSUM bank. start=True
                            # clears has_written for the WHOLE bank, so only
                            # the first matmul touching the bank may set it;
                            # the other region's first write overwrites via
                            # the cleared has_written bit instead.
                            nc.tensor.matmul(
                                psums[m // 2][:, (m % 2) * BL:(m % 2 + 1) * BL],
                                lt[:, m * P:(m + 1) * P],
                                mv,
                                start=(k == 0 and m % 2 == 0),
                                stop=(k == KD - 1),
                                skip_group_check=True,
                            )
                return psums

            def phase2(t):
                """x_new = soft(yk + psum), momentum update, in-place state."""
                first = t == 0
                last = t == T - 1
                cm = coefs[t]
                for mp in range(KD // 2):
                    ps = gp.tile([P, 2 * BL], F32, tag="gps")
                    for half in range(2):
                        m = 2 * mp + half
                        out_ap = ps[:, half * BL:(half + 1) * BL]
                        for k in range(KM):
                            nc.tensor.matmul(
                                out_ap,
                                hs_sb[:, k, m * P:(m + 1) * P],
                                rt[:, k, :],
                                start=(k == 0 and half == 0),
                                stop=(k == KM - 1),
                                skip_group_check=True,
                            )
                    sl = slice(2 * mp, 2 * mp + 2)
                    psp = ps.rearrange("p (a f) -> p a f", f=BL)
                    if not first:
                        # t = yk + G/L, in place in PSUM
                        nc.vector.tensor_tensor(psp, yk[:, sl, :], psp, Alu.add)
                    p_t = tp.tile([P, 2, BL], F32, tag="p_t")
                    n_t = tp.tile([P, 2, BL], F32, tag="n_t")
                    nc.scalar.activation(p_t[:], psp, RELU, bias=negth[:], scale=1.0)
                    nc.scalar.activation(n_t[:], psp, RELU, bias=negth[:], scale=-1.0)
                    if first or last:
                        # first: c_0 == 0 so yk = x_new; last: yk_T unused.
                        nc.vector.tensor_tensor(xo[:, sl, :], p_t[:], n_t[:], Alu.subtract)
                        if not last:
                            nc.scalar.copy(yk[:, sl, :], xo[:, sl, :])
                            nc.vector.tensor_copy(out=ykb[:, sl, :], in_=xo[:, sl, :])
                        else:
                            # bf16(x_new) for the final z = H @ x_new pass
                            nc.vector.tensor_copy(out=ykb[:, sl, :], in_=xo[:, sl, :])
                    else:
                        xs = xp.tile([P, 2, BL], F32, tag="xs")
                        nc.scalar.mul(xs[:], xo[:, sl, :], cm)
                        nc.vector.tensor_tensor(xo[:, sl, :], p_t[:], n_t[:], Alu.subtract)
                        nc.vector.scalar_tensor_tensor(
                            yk[:, sl, :], xo[:, sl, :], 1.0 + cm, xs[:],
                            Alu.mult, Alu.subtract,
                        )
                        # bf16 shadow of yk for the next residual matmul,
                        # emitted right after so next phase 1 starts early
                        nc.vector.tensor_copy(out=ykb[:, sl, :], in_=yk[:, sl, :])

            # Diagnostic knobs (timing experiments only; default off).
            skip_p1 = os.environ.get("KBENCH_SKIP_P1", "0") == "1"
            skip_p2 = os.environ.get("KBENCH_SKIP_P2", "0") == "1"
            reps = int(os.environ.get("KBENCH_REPS", "0"))

            def body():
                # t = 0: yk == 0 so r = x directly.
                nc.vector.tensor_copy(out=rt[:], in_=xt_sb[:])
                phase2(0)
                for t in range(1, T):
                    if not skip_p1:
                        psums = residual_matmuls()
                        for j in range(KM // 2):
                            nc.vector.tensor_tensor(
                                rt[:, 2 * j:2 * j + 2, :],
                                xt_sb[:, 2 * j:2 * j + 2, :],
                                psums[j].rearrange("p (a f) -> p a f", f=BL),
                                Alu.subtract,
                            )
                    if not skip_p2:
                        phase2(t)

                # z = H @ x_new (ykb holds bf16(x_new) now); xt_sb is dead
                # and reused as the z staging buffer.
                psums = residual_matmuls()
                for j in range(KM // 2):
                    nc.vector.tensor_copy(
                        out=xt_sb[:, 2 * j:2 * j + 2, :],
                        in_=psums[j].rearrange("p (a f) -> p a f", f=BL),
                    )
                nc.sync.dma_start(out=zt.rearrange("(o p) f -> p o f", p=P), in_=xt_sb[:])
                nc.sync.dma_start(out=xnt.rearrange("(o p) f -> p o f", p=P), in_=xo[:])

            if reps > 0:
                with tc.For_i(0, reps):
                    body()
            else:
                body()

    nc.finalize()
    return nc


def _get_nc(T):
    nc = _nc_cache.get(T)
    if nc is None:
        nc = _build(T)
        _nc_cache[T] = nc
    return nc


def kernel(x=None, H=None, T=None, trace=False, **_ignored):
    x = np.asarray(x, dtype=np.float32)
    H = np.asarray(H, dtype=np.float32)
    T = int(np.asarray(T))
    assert x.shape == (B, DIN, 1), x.shape
    assert H.shape == (DIN, DENC), H.shape

    if T <= 0:
        z = np.zeros((B, DIN, 1), np.float32)
        xn = np.zeros((B, DENC, 1), np.float32)
        return z, xn

    nc = _get_nc(T)

    x2 = x[:, :, 0]                                    # [B, D_in]
    htb_host = np.ascontiguousarray(H.T.astype(ml_dtypes.bfloat16))
    hs_host = np.ascontiguousarray(
        (H * np.float32(1.0 / L_CONST)).astype(ml_dtypes.bfloat16)
    )                                                  # [D_in, D_enc] bf16

    in_maps = []
    for c in range(N_CORES):
        xt_c = np.ascontiguousarray(x2[c * BL:(c + 1) * BL].T)  # [D_in, BL]
        in_maps.append({"xt": xt_c, "htb": htb_host, "hs": hs_host})

    res = run_bass_kernel_spmd(
        nc, in_maps, core_ids=list(range(N_CORES)), trace=trace
    )

    z = np.empty((B, DIN, 1), np.float32)
    xn = np.empty((B, DENC, 1), np.float32)
    for c in range(N_CORES):
        z[c * BL:(c + 1) * BL, :, 0] = res.results[c]["zt"].T
        xn[c * BL:(c + 1) * BL, :, 0] = res.results[c]["xnt"].T

    if trace:
        kernel.last_results = res
    return z, xn


kernel.last_results = None
